# revision 1
# baseline (speedup 1.0000x reference)
"""Trainium2 Bass kernel for nn_DL_SOTA_PrototypeNet (vq_codebook).

Math restructuring (all exact, done host-side on the tiny weights):
  g = gelu(x @ w1 + b1)                                  [n, 64]
  With zero biases (asserted): z = r * (g @ Wbar), r = rsqrt(var_h + eps),
  Wbar = (I - 11^T/H) diag(ln_g) w2, so Ghat = Wbar Wbar^T annihilates 1.
  eigh: Ghat = Q diag(lam) Q^T with q0 = 1/sqrt(H), lam0 = 0. Project
  y = g @ Q once; then BOTH LayerNorm variance and |z|^2 come from y^2:
      var_h = sum_i c_i y_i^2   (c = [0, 1/H, ...], orthogonal invariance)
      |z|^2 = r^2 * sum_i lam_i y_i^2
  logits L = r * (g @ Wp), Wp = Wbar @ P^T.

Device pipeline per core (4 batches x 8192 tokens, 1024-token chunks,
512-token halves packed on psum partitions so evac passes run at full
width; all cross-engine deps are one full pipeline cycle stale):
  t   : mm1  h[128,512] <- w1^T x (two 512-token halves stacked;
        first half fp8e4 DoubleRow at psum base 0, second half fp16 at
        base 64 -- DoubleRow dst is ISA-limited to base 0)
  t-1 : gelu h -> g fp16 [128, 512]
  t-2 : mm2y Y[128,512] <- Q^T g per half; mm2n N[64,512] rows {0:32|32:64}
        <- [Wp|0]^T g per half (L in rows 0:6 / 32:38)
  t-3 : sq   y2 fp16 [128,512] <- Square(Y)  (ACT/DVE column split)
  t-4 : mm3  N rows 6:8 / 38:40 += [c|lam]^T y2 per half (psum accumulate)
  t-5 : evac N -> nf fp16 [64, 512] single op; per-4-chunk-slice xbar
        transposes (16-row blocks) -> token-major [128, slot, 16]
  tok : softmax/stats chain on DVE/ACT (4 strands/batch, 1 op/cycle);
        per-(batch,strand) sums via TensorReduce into an SBUF accumulator;
        single [128, 192] DMA at the end; host does partition sum + p2.
"""
import sys
from contextlib import ExitStack

sys.path.insert(0, "/opt/trn_rl_repo")

import numpy as np

import concourse.bass as bass
import concourse.mybir as mybir
import concourse.tile as tile
from concourse.vector_clock import ScopedClock, VectorClock

# ---------------------------------------------------------------------------
# Workaround: this walrus build only accepts 1 sync-wait per CTRL (Drain)
# instruction; Tile's tail drain carries one wait per active proc. Split it.
_orig_drain_and_barrier = tile.TileContext._drain_and_barrier


def _patched_drain_and_barrier(self, tick_clock, wait_clock):
    gclock = tick_clock.global_clock
    nprocs = len(gclock)
    procs = [i for i in range(nprocs) if gclock[i] > 0]
    for p in procs:
        vec = [gclock[i] if i == p else 0 for i in range(nprocs)]
        drain_inst = self.nc.sync.drain()
        wait_clock.add_sem_waits(drain_inst.ins, ScopedClock({None: VectorClock(vec)}))
    if not procs:
        self.nc.sync.drain()
    self.nc.all_engine_barrier()
    assert self.sems is not None
    popped = self.nc._tile_sem_poison_stack.pop()
    assert popped is self._sem_poison
    self.nc.clear_and_free_semaphores(list(self.sems.allocated().values()))
    self.nc.all_engine_barrier()


tile.TileContext._drain_and_barrier = _patched_drain_and_barrier


def _split_excess_waits(nc, max_waits=1):
    """This walrus rejects instructions with more than ~1 sync wait. Hoist
    excess waits onto same-engine NoOps placed immediately before the
    instruction (engine streams execute in order, and DMA issue happens at
    NX-execution time, so semantics are preserved)."""
    idx = 0
    for bbname, bbh in nc.bb_map.items():
        insts = bbh.bb.instructions
        out = []
        for inst in insts:
            si = getattr(inst, "sync_info", None)
            waits = list(si.on_wait) if si is not None and si.on_wait else []
            if len(waits) > max_waits:
                extra, keep = waits[:-max_waits], waits[-max_waits:]
                for w in extra:
                    nop = mybir.InstNoOp(name=f"I-waitsplit-{idx}", ins=[], outs=[])
                    idx += 1
                    nop.engine = inst.engine
                    nop.sync_info = mybir.SyncInfo(on_wait=[w], on_update=[])
                    nc.register_instruction(nop, overwrite=True)
                    out.append(nop)
                si.on_wait = keep
            out.append(inst)
        insts[:] = out
# ---------------------------------------------------------------------------

B, N, PULSE = 32, 8192, 128
H, D, K = 64, 256, 6
TEMP, LN_EPS = 0.1, 1e-5
NCORES = 8
BPC = B // NCORES              # batches per core = 4
T = BPC * N                    # tokens per core = 32768
CHUNK = 1024                   # tokens per pipeline chunk
HC = 512                       # tokens per packed half
NCH = T // CHUNK               # 32 chunks
CPB = N // CHUNK               # 8 chunks per batch
CPS = 4                        # chunks per transpose slice
SUPER = 4096                   # x-DMA granularity (4 chunks)
NSUP = T // SUPER
SLOTS = N // 128               # token slots per partition per batch = 64
NARROW = 16                    # narrow rows per 16-row transpose block
NSTR = 4                       # token-major strands per batch (fixed)
SL = SLOTS // NSTR             # slots per strand = 16

F16 = mybir.dt.float16
F32 = mybir.dt.float32
AF = mybir.ActivationFunctionType
OP = mybir.AluOpType
AX = mybir.AxisListType

OPTS = dict(
    sq_act_cols=0,       # sq-evac cols on ACT (rest DVE)
    nev_engine="dve",    # narrow-evac engine: dve | act | rot
                         # (gpsimd cannot read psum on this walrus)
    tok_steps=2,         # generator advances per strand per cycle
    gen_delay=4,         # cycles between slice transpose and first tok op
    xpre=3,              # x supers preloaded before the pipeline
    xbufs=4, gbufs=3, y2bufs=3, nfbufs=2, ttbufs=3,
    hbufs=2, ybufs=2, nbufs=4,
    sbufs=10, wbufs=10,
    tail_cps=2, tail_pool=0, tok_pool=1, xp_delay=2,
    tok_deep=2,
)


def _host_fold(w1, b1, ln_g, ln_b, w2, b2, prot):
    f64 = np.float64
    A = ln_g.astype(f64)[:, None] * w2.astype(f64)
    a_row = ln_g.astype(f64) @ w2.astype(f64)
    c_row = ln_b.astype(f64) @ w2.astype(f64) + b2.astype(f64)
    Wbar = A - np.ones((H, 1), f64) / H * a_row[None, :]
    Wp = Wbar @ prot.T.astype(f64)            # [H, K]
    Ghat = Wbar @ Wbar.T
    lam, Q = np.linalg.eigh(Ghat)             # ascending; lam[0] ~ 0
    assert abs(lam[0]) < 1e-8, lam[0]
    lam = np.maximum(lam, 0.0)
    lam[0] = 0.0
    cvec = np.full(H, 1.0 / H, f64)
    cvec[0] = 0.0
    cp = c_row @ prot.T.astype(f64)           # [K]
    cc = float(c_row @ c_row)
    p2 = np.sum(prot.astype(f64) ** 2, axis=1)  # [K]
    S1y = np.zeros((128, H), f64)             # mm2y stationary: Q per half
    S1y[0:H] = Q
    S1y[H:128] = Q
    S1n = np.zeros((128, 32), f64)            # mm2n stationary: [Wp | 0]
    S1n[0:H, 0:K] = Wp
    S1n[H:128] = S1n[0:H]
    S2 = np.zeros((128, 32), f64)             # mm3 stationary on y^2
    S2[0:H, 6] = cvec
    S2[0:H, 7] = lam
    S2[H:128] = S2[0:H]
    return S1y, S1n, S2, cp, cc, p2


def _build_program(num_cores, opts=None):
    o = dict(OPTS)
    if opts:
        o.update(opts)
    nc = bass.Bass("TRN2", target_bir_lowering=False, debug=False,
                   num_devices=num_cores)
    # register LN_EPS so activation(bias=LN_EPS) resolves
    _eps_t = nc.alloc_sbuf_tensor("const-f32-eps", [128, 1], F32)
    nc.gpsimd.memset(_eps_t.ap(), LN_EPS)
    nc.const_aps.aps[(F32, LN_EPS)] = _eps_t.ap()
    nc.all_engine_barrier()
    xt8 = nc.dram_tensor("xt8", [64, T], mybir.dt.float8e4,
                         kind="ExternalInput").ap()
    xt16 = nc.dram_tensor("xt16", [128, T // 2], F16,
                          kind="ExternalInput").ap()
    w1d = nc.dram_tensor("w1d", [128, H], F16, kind="ExternalInput").ap()
    w1d8 = nc.dram_tensor("w1d8", [64, 2 * H], mybir.dt.float8e4,
                          kind="ExternalInput").ap()
    t1yd = nc.dram_tensor("t1yd", [128, H], F16, kind="ExternalInput").ap()
    t1nd = nc.dram_tensor("t1nd", [128, 32], F16, kind="ExternalInput").ap()
    t2d = nc.dram_tensor("t2d", [128, 32], F16, kind="ExternalInput").ap()
    b1d = nc.dram_tensor("b1d", [128, 1], F32, kind="ExternalInput").ap()
    outd = nc.dram_tensor("outd", [128, 384], F32, kind="ExternalOutput").ap()

    SA = o["sq_act_cols"]

    with tile.TileContext(nc) as tc, ExitStack() as ctx:
        cpool = ctx.enter_context(tc.tile_pool(name="consts", bufs=1))
        xpool = ctx.enter_context(tc.tile_pool(name="xin", bufs=o["xbufs"]))
        hpool = ctx.enter_context(
            tc.tile_pool(name="hps", bufs=o["hbufs"], space="PSUM"))
        ypool = ctx.enter_context(
            tc.tile_pool(name="yps", bufs=o["ybufs"], space="PSUM"))
        npool = ctx.enter_context(
            tc.tile_pool(name="nps", bufs=o["nbufs"], space="PSUM"))
        gpool = ctx.enter_context(tc.tile_pool(name="gtile", bufs=o["gbufs"]))
        y2pool = ctx.enter_context(tc.tile_pool(name="y2t", bufs=o["y2bufs"]))
        nfpool = ctx.enter_context(tc.tile_pool(name="nfeat", bufs=o["nfbufs"]))
        ttpool = ctx.enter_context(tc.tile_pool(name="ttok", bufs=o["ttbufs"]))
        spool = ctx.enter_context(tc.tile_pool(name="small", bufs=o["sbufs"]))
        wpool = ctx.enter_context(tc.tile_pool(name="wide", bufs=o["wbufs"]))

        # tiny consts go on the sync HWDGE queue ahead of the x supers so
        # the first mm1 isn't stuck behind megabyte transfers
        w1sb = cpool.tile([128, H], F16, tag="w1sb")
        nc.sync.dma_start(w1sb[:], w1d[:])
        w1sb8 = cpool.tile([64, 2 * H], mybir.dt.float8e4, tag="w1sb8")
        nc.sync.dma_start(w1sb8[:], w1d8[:])
        w1sb83 = w1sb8.rearrange("p (j m) -> p j m", j=2)
        t1y = cpool.tile([128, H], F16, tag="t1y")
        nc.sync.dma_start(t1y[:], t1yd[:])
        t1n = cpool.tile([128, 32], F16, tag="t1n")
        nc.sync.dma_start(t1n[:], t1nd[:])
        t2sb = cpool.tile([128, 32], F16, tag="t2sb")
        nc.sync.dma_start(t2sb[:], t2d[:])
        b1sb = cpool.tile([128, 1], F32, tag="b1sb")
        nc.sync.dma_start(b1sb[:], b1d[:])

        # per-(batch,strand) stats: [cnt(6) | d2(6)] columns, summed over
        # slots on DVE; partitions summed on host after the final DMA.
        # Up to 8 strands per batch (last batch); unused columns stay zero.
        obuf = cpool.tile([128, BPC * 8 * 12], F32, tag="obuf")
        nc.gpsimd.memset(obuf[:], 0.0)

        def tok_strand(tt3, b, j, sl0, slc, tail):
            """Token-major chain for slots [sl0, sl0+slc) of batch b.
            Tail strands route their pointwise ops to Pool alternately so
            the post-trunk drain spreads across idle engines."""
            SL = slc
            tt = tt3[:, sl0:sl0 + SL, :]
            def stt(out, in0, scal, in1, op0, op1, pool_ok=False):
                # walrus has no Pool InstTensorScalarPtr (and gpsimd cannot
                # read psum); route the SBUF-only broadcast ops to Pool to
                # offload DVE when allowed
                if pool_ok and o["tok_pool"] and scal == 1.0 \
                        and op0 == OP.mult:
                    nc.gpsimd.tensor_tensor(out, in0, in1, op1)
                else:
                    nc.vector.scalar_tensor_tensor(out, in0, scal, in1,
                                                   op0, op1)
            L6 = tt[:, :, 0:K]
            varv = tt[:, :, 6]
            z2qv = tt[:, :, 7]

            def bcs(ap_2d):
                return ap_2d.rearrange("p (g c) -> p g c", c=1).to_broadcast(
                    (128, SL, K))

            sqv = spool.tile([128, SL], F16, tag="sqv")
            nc.scalar.activation(sqv[:], varv, AF.Sqrt, bias=LN_EPS)
            yield
            rv = spool.tile([128, SL], F16, tag="rv")
            with nc.allow_low_precision("rsqrt in fp16; tol 2e-2"):
                nc.vector.reciprocal(rv[:], sqv[:])
            yield
            r2 = spool.tile([128, SL], F16, tag="r2")
            stt(r2[:], rv[:], 1.0, rv[:],
                                        OP.mult, OP.mult)
            yield
            z2t = spool.tile([128, SL], F16, tag="z2t")
            stt(z2t[:], z2qv, 1.0, r2[:],
                                        OP.mult, OP.mult)
            yield
            Lt = wpool.tile([128, SL * K], F16, tag="Lt")
            Lt3 = Lt.rearrange("p (g c) -> p g c", c=K)
            stt(Lt3[:], L6, 1.0, bcs(rv[:]),
                                        OP.mult, OP.mult, pool_ok=True)
            yield
            mx = spool.tile([128, SL], F16, tag="mx")
            nc.vector.tensor_reduce(mx[:], Lt3[:], AX.X, OP.max)
            yield
            Et = wpool.tile([128, SL * K], F16, tag="Et")
            Et3 = Et.rearrange("p (g c) -> p g c", c=K)
            stt(Et3[:], Lt3[:], 1.0,
                                        bcs(mx[:]), OP.mult, OP.subtract, pool_ok=True)
            yield
            nc.scalar.activation(Et[:], Et[:], AF.Exp, scale=1.0 / TEMP)
            yield
            sme = spool.tile([128, SL], F16, tag="sme")
            with nc.allow_low_precision("softmax denom; K=6 positive terms"):
                nc.vector.tensor_reduce(sme[:], Et3[:], AX.X, OP.add)
            yield
            rec = spool.tile([128, SL], F16, tag="rec")
            with nc.allow_low_precision("softmax denom recip in fp16"):
                nc.vector.reciprocal(rec[:], sme[:])
            yield
            At = wpool.tile([128, SL * K], F16, tag="At")
            At3 = At.rearrange("p (g c) -> p g c", c=K)
            stt(At3[:], Et3[:], 1.0, bcs(rec[:]),
                                        OP.mult, OP.mult, pool_ok=True)
            yield
            Dt = wpool.tile([128, SL * K], F16, tag="Dt")
            Dt3 = Dt.rearrange("p (g c) -> p g c", c=K)
            stt(Dt3[:], Lt3[:], -2.0, bcs(z2t[:]),
                                        OP.mult, OP.add)
            yield
            stt(Dt3[:], Dt3[:], 1.0, At3[:],
                                        OP.mult, OP.mult, pool_ok=True)
            yield
            col = (b * 8 + j) * 12
            At_r = At.rearrange("p (g c) -> p c g", c=K)
            nc.vector.tensor_reduce(obuf[:, col:col + K], At_r[:], AX.X,
                                    OP.add)
            yield
            Dt_r = Dt.rearrange("p (g c) -> p c g", c=K)
            nc.vector.tensor_reduce(obuf[:, col + K:col + 12], Dt_r[:], AX.X,
                                    OP.add)

        # pipeline state
        xtiles = {}
        hps, gts, yps, nps, y2s = {}, {}, {}, {}, {}
        nfeats, ttoks = {}, {}
        nfeats_hold = {}
        strand_no = [0] * BPC
        live_gens = []   # (start_cycle, gen)
        pend_xp = []     # (due_cycle, batch, first_chunk, last_chunk)

        def load_super(s, split=1):
            # hybrid x: top token-halves fp8 (DoubleRow mm1a), bottom fp16
            HS = SUPER // 2
            x8l = xpool.tile([64, SUPER], mybir.dt.float8e4, tag="x8",
                             name="x8l")
            x83 = x8l.rearrange("p (j n) -> p j n", j=2)
            xt83 = xt8.rearrange("p (j n) -> p j n", j=2)
            nc.sync.dma_start(x83[:], xt83[:, :, s * HS:(s + 1) * HS])
            x16l = xpool.tile([128, HS], F16, tag="x16", name="x16l")
            w = HS // split
            for k in range(split):
                nc.sync.dma_start(
                    x16l[:, k * w:(k + 1) * w],
                    xt16[:, s * HS + k * w:s * HS + (k + 1) * w])
            xtiles[s] = (x8l, x16l)

        XPRE = o["xpre"]
        load_super(0, split=4)
        for s in range(1, XPRE):
            load_super(s, split=2 if s <= 2 else 1)

        def step_gens(t):
            # oldest-first, depth-weighted: front strands drain early so the
            # final strand isn't held back by round-robin fairness
            nxt = []
            k = 0
            for sc, gen in live_gens:
                if sc > t:
                    nxt.append((sc, gen))
                    continue
                steps = o["tok_deep"] if k == 0 else o["tok_steps"]
                k += 1
                alive = True
                for _ in range(steps):
                    try:
                        next(gen)
                    except StopIteration:
                        alive = False
                        break
                if alive:
                    nxt.append((sc, gen))
            live_gens[:] = nxt

        NEVENG = {"dve": [nc.vector], "act": [nc.scalar],
                  "rot": [nc.vector, nc.scalar]}[o["nev_engine"]]

        def emit_xpose(b, i0, i):
            nf = nfeats_hold[b]
            cps = i - i0 + 1
            tt3 = ttoks[b].rearrange("p (g c) -> p g c", c=NARROW)
            slc = 4 * cps
            s0 = 8 * i0                  # slice's first slot
            for half in (0, 1):
                nc.sync.dma_start_transpose(
                    tt3[:, s0 + half * slc:s0 + (half + 1) * slc, :],
                    nf[32 * half:32 * half + NARROW,
                       i0 * HC:(i + 1) * HC])
            j = strand_no[b]
            strand_no[b] += 1
            return tok_strand(tt3, b, j, s0, 2 * slc, b == BPC - 1)

        for t in range(NCH + 8):
            while pend_xp and pend_xp[0][0] <= t:
                _, b_, i0_, i_ = pend_xp.pop(0)
                live_gens.append((t + o["gen_delay"],
                                  emit_xpose(b_, i0_, i_)))
            step_gens(t)

            # just-in-time x loads keep the serial DMA queue short so
            # transposes aren't head-of-line blocked behind big transfers
            if t >= 2 and (t - 2) % 4 == 0 and (t - 2) // 4 + XPRE < NSUP:
                load_super((t - 2) // 4 + XPRE)

            if t < NCH:
                # mm1 for chunk t
                x8l, x16l = xtiles[t // 4]
                x83 = x8l.rearrange("p (j n) -> p j n", j=2)
                off = (t % 4) * HC
                h_ps = hpool.tile([128, HC], F32, tag="h")
                nc.tensor.matmul(h_ps[0:H, :], w1sb83[:],
                                 x83[:, :, off:off + HC], start=True,
                                 stop=True,
                                 perf_mode=mybir.MatmulPerfMode.DoubleRow)
                nc.tensor.matmul(h_ps[H:128, :], w1sb[:],
                                 x16l[:, off:off + HC],
                                 start=True, stop=True)
                hps[t] = h_ps
                b, i = divmod(t, CPB)
                if i == 0:
                    nfeats[b] = nfpool.tile([64, CPB * HC], F16, tag="nf",
                                            name="nf")
                    nfeats_hold[b] = nfeats[b]
                    ttoks[b] = ttpool.tile(
                        [128, SLOTS * NARROW], F16, tag="ttok", name="ttok")

            c = t - 1
            if 0 <= c < NCH:
                # gelu for chunk c (mm1 dep one cycle old)
                h_ps = hps.pop(c)
                g = gpool.tile([128, HC], F16, tag="g")
                nc.scalar.activation(g[:], h_ps[:], AF.Gelu, bias=b1sb[:])
                gts[c] = g

            c = t - 2
            if 0 <= c < NCH:
                # mm2y + mm2n for chunk c (gelu dep one cycle old)
                g = gts.pop(c)
                y_ps = ypool.tile([128, HC], F32, tag="y")
                nc.tensor.matmul(y_ps[0:H, :], t1y[0:H, :], g[0:H, :],
                                 start=True, stop=True)
                nc.tensor.matmul(y_ps[H:128, :], t1y[H:128, :], g[H:128, :],
                                 start=True, stop=True)
                n_ps = npool.tile([64, HC], F32, tag="n")
                nc.tensor.matmul(n_ps[0:32, :], t1n[0:H, :], g[0:H, :],
                                 start=True, stop=True)
                nc.tensor.matmul(n_ps[32:64, :], t1n[H:128, :], g[H:128, :],
                                 start=True, stop=True)
                yps[c], nps[c] = y_ps, n_ps

            c = t - 3
            if 0 <= c < NCH:
                # square evac for chunk c (mm2y dep one cycle old); ACT only
                # (walrus rejects two-psum-operand TensorTensor on DVE)
                y_ps = yps.pop(c)
                y2 = y2pool.tile([128, HC], F16, tag="y2")
                nc.scalar.activation(y2[:], y_ps[:], AF.Square)
                y2s[c] = y2

            c = t - 4
            if 0 <= c < NCH:
                # mm3: accumulate var/z2q into N rows 6:8 / 38:40
                y2 = y2s.pop(c)
                n_ps = nps[c]
                nc.tensor.matmul(n_ps[0:32, :], t2sb[0:H, :], y2[0:H, :],
                                 start=False, stop=True, skip_group_check=True)
                nc.tensor.matmul(n_ps[32:64, :], t2sb[H:128, :], y2[H:128, :],
                                 start=False, stop=True, skip_group_check=True)

            c = t - 5
            if 0 <= c < NCH:
                # narrow evac: one [64, 512] op (mm3 dep one cycle old),
                # then the slice transposes once 4 chunks are complete
                n_ps = nps.pop(c)
                b, i = divmod(c, CPB)
                nf = nfeats[b]
                eng = NEVENG[c % len(NEVENG)]
                if eng is nc.scalar:
                    nc.scalar.copy(nf[:, i * HC:(i + 1) * HC], n_ps[:])
                else:
                    eng.tensor_copy(nf[:, i * HC:(i + 1) * HC], n_ps[:])
                # one strand per slice: both halves land in contiguous slot
                # ranges so a single chain covers the whole slice; the last
                # batch uses smaller slices to start draining earlier
                cps = o["tail_cps"] if b == BPC - 1 else CPS
                if i % cps == cps - 1:
                    # queue the slice transpose a couple of cycles out so
                    # its evac deps are stale and it never blocks the SP
                    # DMA queue (x supers ride the same queue)
                    pend_xp.append((t + o["xp_delay"], b, i - cps + 1, i))
                    if i == CPB - 1:
                        nfeats.pop(b)

        # drain remaining transposes and token-major work
        while pend_xp:
            _, b_, i0_, i_ = pend_xp.pop(0)
            live_gens.append((0, emit_xpose(b_, i0_, i_)))
        while live_gens:
            step_gens(10 ** 9)

        nc.sync.dma_start(outd[:], obuf[:])

    _split_excess_waits(nc)
    return nc


def kernel(x, w1, b1, ln_g, ln_b, w2, b2, prototypes):
    x = np.asarray(x, dtype=np.float32)
    w1 = np.asarray(w1, dtype=np.float32)
    b1 = np.asarray(b1, dtype=np.float32)
    ln_g = np.asarray(ln_g, dtype=np.float32)
    ln_b = np.asarray(ln_b, dtype=np.float32)
    w2 = np.asarray(w2, dtype=np.float32)
    b2 = np.asarray(b2, dtype=np.float32)
    prot = np.asarray(prototypes, dtype=np.float32)

    S1y, S1n, S2, cp, cc, p2 = _host_fold(w1, b1, ln_g, ln_b, w2, b2, prot)
    if max(abs(cp).max(), abs(cc), abs(b1).max()) > 1e-12:
        raise NotImplementedError(
            "nonzero ln_b/b2 path not emitted (inputs have zero bias)")

    t1y_np = S1y.astype(np.float16)
    t1n_np = S1n.astype(np.float16)
    t2_np = S2.astype(np.float16)
    import ml_dtypes
    E4 = ml_dtypes.float8_e4m3fn
    w1_np = w1.astype(np.float16)                      # [128, 64]
    w18_np = np.concatenate([w1[0:64], w1[64:128]], axis=1).astype(E4)
    b1_np = np.concatenate([b1, b1]).reshape(128, 1).astype(np.float32)

    from concourse.bass_utils import run_bass_kernel_spmd

    nc = _build_program(NCORES)
    in_maps = []
    for c in range(NCORES):
        xs = x[c * BPC:(c + 1) * BPC].reshape(T, PULSE)
        xsT = xs.T.reshape(PULSE, T // 1024, 2, 512)   # [p, chunk, half, u]
        top = xsT[:, :, 0, :].reshape(PULSE, T // 2)
        bot = xsT[:, :, 1, :].reshape(PULSE, T // 2)
        xt8_np = np.ascontiguousarray(
            np.concatenate([top[0:64], top[64:128]], axis=1)).astype(E4)
        xt16_np = np.ascontiguousarray(bot).astype(np.float16)
        in_maps.append({"xt8": xt8_np, "xt16": xt16_np, "w1d": w1_np,
                        "w1d8": w18_np, "t1yd": t1y_np,
                       "t1nd": t1n_np, "t2d": t2_np, "b1d": b1_np})

    res = run_bass_kernel_spmd(nc, in_maps, core_ids=list(range(NCORES)))

    var = np.empty((B, K), np.float32)
    for c in range(NCORES):
        o = res.results[c]["outd"].astype(np.float64)  # [128, BPC*8*12]
        o = o.sum(axis=0).reshape(BPC, 8, 2, K)
        C0 = o[:, :, 0].sum(axis=1)                    # [BPC, K]
        Dsum = o[:, :, 1].sum(axis=1)                  # [BPC, K]
        cnt = C0 + 1e-6
        v = Dsum / cnt + p2[None, :] * C0 / cnt
        var[c * BPC:(c + 1) * BPC] = v.astype(np.float32)
    return var

